# revision 17
# baseline (speedup 1.0000x reference)
"""Trainium2 Bass kernel for nn_AttentionLayer_s (sparse attention via
per-memory-node top-k selection), SPMD over 8 NeuronCores.

Sharding: batch dimension (B=16 -> 2 per core); weights replicated; no
cross-core communication. Per (b,t) tile the kernel computes projections,
node-selection scores, exact top-50 masks (max8/match_replace rounds), and
a mask-weighted dense attention: E~ = exp(k q^T/4) tiles (j-major), then per
memory node U = E~^T (mask*[v|1]), out += mask * U[:,:16]/U[:,16]; finally
agg/(cnt+eps), head merge and output projection.
"""
import sys

sys.path.insert(0, '/opt/trn_rl_repo')

import numpy as np

from concourse import bass, mybir
from concourse import tile as _tile
from concourse.vector_clock import ScopedClock

B, T, N, D = 16, 12, 1024, 128
H = 8
HD = 16
TOPK = 50
M = 20
NCORES = 8
BS = B // NCORES

F32 = mybir.dt.float32
BF16 = mybir.dt.bfloat16
NEG = -1e30
AX = mybir.AxisListType.X
AOP = mybir.AluOpType
AF = mybir.ActivationFunctionType


# ---------------------------------------------------------------- tile patches
def _drain_and_barrier(self, tick_clock, wait_clock):
    nc = self.nc
    drain_inst = nc.sync.drain()
    wait_clock.add_sem_waits(
        drain_inst.ins, ScopedClock({None: tick_clock.global_clock})
    )
    si = drain_inst.ins.sync_info
    if si is not None and len(si.on_wait) > 1:
        waits = list(si.on_wait)
        si.on_wait = waits[:1]
        for w in waits[1:]:
            nop = nc.sync.nop(nofuse=True)
            nop.ins.sync_info = mybir.SyncInfo(on_wait=[w], on_update=[])
    nc.all_engine_barrier()
    assert self.sems is not None
    popped = nc._tile_sem_poison_stack.pop()
    assert popped is self._sem_poison
    nc.clear_and_free_semaphores(list(self.sems.allocated().values()))
    nc.all_engine_barrier()


_tile.TileContext._drain_and_barrier = _drain_and_barrier


def split_waits(nc, max_waits=1):
    """This env's walrus rejects >1 sem wait per instruction; move excess
    waits onto same-engine NoOps inserted before the instruction."""
    for f in nc.m.functions:
        for bb in f.blocks:
            out = []
            changed = False
            for inst in bb.instructions:
                si = inst.sync_info
                if si is not None and len(si.on_wait) > max_waits:
                    waits = list(si.on_wait)
                    si.on_wait = waits[-max_waits:]
                    for i, w in enumerate(waits[:-max_waits]):
                        nop = mybir.InstNoOp(
                            name=f"{inst.name}-wsp{i}", ins=[], outs=[])
                        nop.engine = inst.engine
                        nop.sync_info = mybir.SyncInfo(on_wait=[w], on_update=[])
                        nc.register_instruction(nop, overwrite=True)
                        out.append(nop)
                        changed = True
                out.append(inst)
            if changed:
                bb.instructions = out


# ---------------------------------------------------------------- builder
def build_kernel():
    from concourse.tile import TileContext
    from concourse.masks import make_identity

    nc = bass.Bass()
    dp = {}
    for nm in ("query", "key", "value"):
        dp[nm] = nc.declare_dram_parameter(nm, [BS, T, N, D], F32, isOutput=False)
    for nm in ("Wq", "Wk", "Wv", "Wo0", "Wo1", "Wo2", "Wo3"):
        dp[nm] = nc.declare_dram_parameter(nm, [D, D], F32, isOutput=False)
    for nm in ("bq", "bk", "bv", "bo"):
        dp[nm] = nc.declare_dram_parameter(nm, [D, 1], F32, isOutput=False)
    for nm in ("embq", "embk"):
        dp[nm] = nc.declare_dram_parameter(nm, [64, 80], F32, isOutput=False)
    dp["onesblk"] = nc.declare_dram_parameter("onesblk", [80, 4], F32,
                                              isOutput=False)
    out_ext = nc.declare_dram_parameter("out", [BS, T, N, D], F32, isOutput=True)

    from contextlib import ExitStack
    def mm512(out, lhsT, rhs, start, stop):
        n = rhs.shape[-1]
        for o in range(0, n, 512):
            e = min(o + 512, n)
            nc.tensor.matmul(out=out[:, o:e], lhsT=lhsT, rhs=rhs[:, o:e],
                             start=start, stop=stop)

    with TileContext(nc) as tc, ExitStack() as es:
        cpool = es.enter_context(tc.tile_pool(name="const", bufs=1))
        ident = cpool.tile([128, 128], F32)
        make_identity(nc, ident[:])
        identb = cpool.tile([128, 128], BF16, tag="identb")
        nc.vector.tensor_copy(identb[:], ident[:])
        w_sb = {}
        for nm in ("Wq", "Wk", "Wv", "Wo0", "Wo1", "Wo2", "Wo3"):
            w = cpool.tile([D, D], BF16, tag=f"w{nm}")
            nc.gpsimd.dma_start(out=w[:], in_=dp[nm][:])
            w_sb[nm] = w
        wf_sb = {}
        for nm in ("Wq", "Wk"):
            wf = cpool.tile([D, D], F32, tag=f"wf{nm}")
            nc.sync.dma_start(out=wf[:], in_=dp[nm][:])
            wf_sb[nm] = wf
        b_sb = {}
        for nm in ("bq", "bk", "bv", "bo"):
            bb_ = cpool.tile([D, 1], F32, tag=f"b{nm}")
            nc.sync.dma_start(out=bb_[:], in_=dp[nm][:])
            b_sb[nm] = bb_
        emb_sb = {}
        for nm in ("embq", "embk"):
            e = cpool.tile([128, 80], F32, tag=f"e{nm}")
            nc.sync.dma_start(out=e[0:64, :], in_=dp[nm][:])
            nc.sync.dma_start(out=e[64:128, :], in_=dp[nm][:])
            emb_sb[nm] = e
        onesblk = cpool.tile([80, 4], BF16, tag="onesblk")
        nc.gpsimd.dma_start(out=onesblk[:], in_=dp["onesblk"][:])

        xpool = es.enter_context(tc.tile_pool(name="x", bufs=1))
        qkvpool = es.enter_context(tc.tile_pool(name="qkv", bufs=2))
        spool = es.enter_context(tc.tile_pool(name="s", bufs=2))
        epool = es.enter_context(tc.tile_pool(name="e", bufs=2))
        apool = es.enter_context(tc.tile_pool(name="a", bufs=2))
        pbig = es.enter_context(tc.tile_pool(name="pbig", bufs=1, space="PSUM"))
        peps = es.enter_context(tc.tile_pool(name="peps", bufs=1, space="PSUM"))
        psm = es.enter_context(tc.tile_pool(name="psm", bufs=2, space="PSUM"))
        pat = es.enter_context(tc.tile_pool(name="pat", bufs=2, space="PSUM"))

        for b in range(BS):
            for t in range(T):
                # ---------- projections (transposed layout, bf16)
                qkvT = {}
                for nm, wname, bname in (("query", "Wq", "bq"),
                                         ("key", "Wk", "bk"),
                                         ("value", "Wv", "bv")):
                    x = xpool.tile([128, 8, 128], F32, tag="x")
                    nc.sync.dma_start(
                        out=x[:],
                        in_=dp[nm][b, t].rearrange("(o p) d -> p o d", p=128))
                    xT_ps = pbig.tile([128, 1024], F32, tag="big")
                    for i in range(8):
                        nc.tensor.transpose(
                            out=xT_ps[:, i * 128:(i + 1) * 128],
                            in_=x[:, i, :], identity=ident[:])
                    if nm == "value":
                        xT = xpool.tile([128, 1024], BF16, tag="xt")
                        nc.scalar.activation(xT[:], xT_ps[:], AF.Copy)
                        pT_ps = pbig.tile([128, 1024], F32, tag="big")
                        mm512(pT_ps[:], w_sb[wname][:], xT[:], True, True)
                    else:
                        xTf = xpool.tile([128, 1024], F32, tag="xtf")
                        nc.scalar.activation(xTf[:], xT_ps[:], AF.Copy)
                        pT_ps = pbig.tile([128, 1024], F32, tag="big")
                        mm512(pT_ps[:], wf_sb[wname][:], xTf[:], True, True)
                        pf = qkvpool.tile([128, 1024], F32, tag=f"pf{nm}")
                        nc.vector.tensor_scalar(pf[:], pT_ps[:], b_sb[bname][:],
                                                scalar2=None, op0=AOP.add)
                        qkvF = getattr(nc, "_qkvF", {})
                        qkvF[nm] = pf
                        nc._qkvF = qkvF
                    pT = qkvpool.tile([128, 1024], BF16, tag=f"p{nm}")
                    nc.vector.tensor_scalar(pT[:], pT_ps[:], b_sb[bname][:],
                                            scalar2=None, op0=AOP.add)
                    qkvT[nm] = pT
                qkvL = {}
                for nm in ("query", "key", "value"):
                    lo = qkvpool.tile([16, 8, 1024], BF16, tag=f"lo{nm}", bufs=1)
                    for h in range(H):
                        nc.scalar.dma_start(
                            out=lo[:, h, :],
                            in_=qkvT[nm][h * HD:(h + 1) * HD, :])
                    qkvL[nm] = lo

                # ---------- scores + exact top-50 masks + counts
                maskTs = []
                rcntTs = []
                for g in range(2):
                    sc_ps = pbig.tile([80, 1024], F32, tag="big")
                    mm512(sc_ps[:], emb_sb["embq"][g * 64:(g + 1) * 64, :],
                          nc._qkvF["query"][g * 64:(g + 1) * 64, :], True, False)
                    mm512(sc_ps[:], emb_sb["embk"][g * 64:(g + 1) * 64, :],
                          nc._qkvF["key"][g * 64:(g + 1) * 64, :], False, True)
                    sc = spool.tile([80, 1024], F32, tag="sc")
                    nc.vector.tensor_copy(sc[:], sc_ps[:])
                    mx = spool.tile([80, 8], F32, tag="mx")
                    for r in range(7):
                        nc.vector.max(out=mx[:], in_=sc[:])
                        if r == 6:
                            nc.vector.memset(mx[:, 2:8], NEG)
                        nc.vector.match_replace(out=sc[:], in_to_replace=mx[:],
                                                in_values=sc[:], imm_value=NEG)
                    mask = spool.tile([80, 1024], BF16, tag="mask")
                    nc.vector.tensor_scalar(mask[:], sc[:], float(NEG),
                                            scalar2=None, op0=AOP.is_equal)
                    cnt_ps = pbig.tile([4, 1024], F32, tag="big")
                    mm512(cnt_ps[:], onesblk[:], mask[:], True, True)
                    cnt = spool.tile([4, 1024], F32, tag="cnt")
                    nc.vector.tensor_scalar(cnt[:], cnt_ps[:], 1e-14,
                                            scalar2=None, op0=AOP.add)
                    mT_ps = psm.tile([128, 8 * 80], BF16, tag="small")
                    for i in range(8):
                        nc.tensor.transpose(
                            out=mT_ps[:, i * 80:(i + 1) * 80],
                            in_=mask[:, i * 128:(i + 1) * 128],
                            identity=identb[0:80, 0:80])
                    mT = spool.tile([128, 8, 80], BF16, tag="maskT")
                    nc.scalar.activation(
                        mT[:], mT_ps[:].rearrange("p (o c) -> p o c", o=8),
                        AF.Copy)
                    maskTs.append(mT)
                    cT_ps = psm.tile([128, 8 * 4], F32, tag="small")
                    for i in range(8):
                        nc.tensor.transpose(
                            out=cT_ps[:, i * 4:(i + 1) * 4],
                            in_=cnt[:, i * 128:(i + 1) * 128],
                            identity=ident[0:4, 0:4])
                    rcT = spool.tile([128, 8, 4], F32, tag="rcntT")
                    nc.vector.reciprocal(
                        rcT[:], cT_ps[:].rearrange("p (o c) -> p o c", o=8))
                    rcntTs.append(rcT)

                # ---------- per-head masked-dense attention
                aggT_g = [None] * 4
                aggqs = [None] * 4
                for h in range(H):
                    g, hh = divmod(h, 4)
                    qt, qh2 = divmod(h, 2)
                    if qh2 == 0:
                        aggT_g[qt] = pat.tile([128, 1024], BF16, tag="atps", name=f"atps{qt}")
                    qh = qkvL["query"][:, h, :]
                    kh = qkvL["key"][:, h, :]
                    vh = qkvL["value"][:, h, :]
                    etiles = []
                    for jt in range(8):
                        e_ps = peps.tile([128, 1024], F32, tag="eps")
                        mm512(e_ps[:], kh[:, jt * 128:(jt + 1) * 128], qh[:],
                              True, True)
                        et = epool.tile([128, 1024], BF16, tag=f"et{jt}", bufs=1)
                        nc.scalar.activation(et[:], e_ps[:], AF.Exp, scale=0.25)
                        etiles.append(et)
                    # v-ext (j-part): (128, 8, 17) = [v | 1]
                    vx_ps = psm.tile([128, 8 * 16], BF16, tag="small")
                    for jt in range(8):
                        nc.tensor.transpose(
                            out=vx_ps[:, jt * 16:(jt + 1) * 16],
                            in_=vh[:, jt * 128:(jt + 1) * 128],
                            identity=identb[0:16, 0:16])
                    vx = epool.tile([128, 8, 17], BF16, tag="vx")
                    nc.vector.tensor_copy(
                        vx[:, :, 0:16],
                        vx_ps[:].rearrange("p (o c) -> p o c", o=8))
                    nc.vector.memset(vx[:, :, 16:17], 1.0)
                    # masked v for all 20 memory nodes: (128, 8, 20, 17)
                    mT = maskTs[g]
                    mv = epool.tile([128, 8, M, 17], BF16, tag="mv", bufs=1)
                    for m in range(M):
                        row = hh * 20 + m
                        nc.gpsimd.tensor_tensor(
                            out=mv[:, :, m, :], in0=vx[:],
                            in1=mT[:, :, row:row + 1].to_broadcast([128, 8, 17]),
                            op=AOP.mult)
                    agg = apool.tile([128, 8, 16], F32, tag="agg")
                    for nt in range(8):
                        u_ps = psm.tile([128, M * 17], F32, tag="small", name="u_ps")
                        for jt in range(8):
                            nc.tensor.matmul(
                                out=u_ps[:],
                                lhsT=etiles[jt][:, nt * 128:(nt + 1) * 128],
                                rhs=mv[:, jt, :, :].rearrange("p m c -> p (m c)"),
                                start=(jt == 0), stop=(jt == 7))
                        upv = u_ps[:].rearrange("p (m c) -> p m c", m=M)
                        rz = spool.tile([128, M, 1], F32, tag="rz")
                        nc.vector.reciprocal(rz[:], upv[:, :, 16:17])
                        rzm = spool.tile([128, M, 1], F32, tag="rzm")
                        nc.vector.tensor_tensor(
                            out=rzm[:], in0=rz[:],
                            in1=mT[:, nt, hh * 20:(hh + 1) * 20].unsqueeze(-1),
                            op=AOP.mult)
                        tmp = spool.tile([128, M, 16], F32, tag="utmp")
                        nc.vector.tensor_tensor(
                            out=tmp[:], in0=upv[:, :, 0:16],
                            in1=rzm[:].to_broadcast([128, M, 16]),
                            op=AOP.mult)
                        # sum over m (innermost via transposed view)
                        nc.vector.tensor_reduce(
                            out=agg[:, nt, :],
                            in_=tmp[:].transpose([0, 2, 1]),
                            axis=AX, op=AOP.add)
                    # divide by cnt
                    nc.vector.tensor_tensor(
                        out=agg[:], in0=agg[:],
                        in1=rcntTs[g][:, :, hh:hh + 1].to_broadcast([128, 8, 16]),
                        op=AOP.mult)
                    agg16 = apool.tile([128, 8, 16], BF16, tag="agg16")
                    nc.scalar.activation(agg16[:], agg[:], AF.Copy)
                    for nt in range(8):
                        nc.tensor.transpose(
                            out=aggT_g[qt][64 * qh2:64 * qh2 + 16,
                                           nt * 128:(nt + 1) * 128],
                            in_=agg16[:, nt, :], identity=identb[:])
                    if qh2 == 1:
                        aggq = apool.tile([128, 1024], BF16, tag="aggq",
                                          name=f"aggq{qt}")
                        nc.vector.memset(aggq[:], 0.0)
                        nc.vector.tensor_copy(aggq[0:16, :],
                                              aggT_g[qt][0:16, :])
                        nc.vector.tensor_copy(aggq[64:80, :],
                                              aggT_g[qt][64:80, :])
                        aggqs[qt] = aggq

                # ---------- output projection + store
                y_ps = pbig.tile([128, 1024], F32, tag="big")
                for qt in range(4):
                    mm512(y_ps[:], w_sb[f"Wo{qt}"][:], aggqs[qt][:],
                          qt == 0, qt == 3)
                yT = apool.tile([128, 1024], F32, tag="yT")
                nc.vector.tensor_scalar(yT[:], y_ps[:], b_sb["bo"][:],
                                        scalar2=None, op0=AOP.add)
                yn_ps = pbig.tile([128, 1024], F32, tag="big")
                for nt in range(8):
                    nc.tensor.transpose(
                        out=yn_ps[:, nt * 128:(nt + 1) * 128],
                        in_=yT[:, nt * 128:(nt + 1) * 128], identity=ident[:])
                yn = apool.tile([128, 8, 128], F32, tag="yn")
                nc.scalar.activation(
                    yn[:], yn_ps[:].rearrange("p (o c) -> p o c", o=8), AF.Copy)
                nc.sync.dma_start(
                    out=out_ext[b, t].rearrange("(o p) d -> p o d", p=128),
                    in_=yn[:])


    split_waits(nc)
    return nc


_NC_CACHE = None


def kernel(**inputs):
    global _NC_CACHE
    from concourse.bass_utils import run_bass_kernel_spmd

    q = np.ascontiguousarray(np.asarray(inputs["query"], np.float32))
    k = np.ascontiguousarray(np.asarray(inputs["key"], np.float32))
    v = np.ascontiguousarray(np.asarray(inputs["value"], np.float32))
    Wq = np.asarray(inputs["Wq"], np.float32)
    Wk = np.asarray(inputs["Wk"], np.float32)
    Wv = np.asarray(inputs["Wv"], np.float32)
    Wo = np.asarray(inputs["Wo"], np.float32)
    bq = np.asarray(inputs["bq"], np.float32).reshape(D, 1)
    bk = np.asarray(inputs["bk"], np.float32).reshape(D, 1)
    bv = np.asarray(inputs["bv"], np.float32).reshape(D, 1)
    bo = np.asarray(inputs["bo"], np.float32).reshape(D, 1)
    emb = np.asarray(inputs["node_emb"], np.float32)

    embq = np.zeros((64, 80), np.float32)
    embk = np.zeros((64, 80), np.float32)
    eq = emb[:, :HD].T
    ek = emb[:, HD:].T
    for hh in range(4):
        embq[hh * 16:(hh + 1) * 16, hh * 20:(hh + 1) * 20] = eq
        embk[hh * 16:(hh + 1) * 16, hh * 20:(hh + 1) * 20] = ek
    # merge-heads layout: _merge_heads puts head h at output dims h*16..h*16+16,
    # i.e. out @ Wo uses Wo rows h*16..+16 for head h. aggT row-blocks sit at
    # partition 32*hh of tile g (heads 0-3 -> WoA, 4-7 -> WoB).
    onesblk_np = np.zeros((80, 4), np.float32)
    for hh in range(4):
        onesblk_np[hh * 20:(hh + 1) * 20, hh] = 1.0
    Wos = [np.zeros((D, D), np.float32) for _ in range(4)]
    for h in range(H):
        qt, qh2 = divmod(h, 2)
        Wos[qt][64 * qh2:64 * qh2 + 16, :] = Wo[h * HD:(h + 1) * HD, :]

    if _NC_CACHE is None:
        _NC_CACHE = build_kernel()
    nc = _NC_CACHE

    maps = []
    for c in range(NCORES):
        maps.append({
            "query": q[c * BS:(c + 1) * BS],
            "key": k[c * BS:(c + 1) * BS],
            "value": v[c * BS:(c + 1) * BS],
            "Wq": Wq, "Wk": Wk, "Wv": Wv,
            "Wo0": Wos[0], "Wo1": Wos[1], "Wo2": Wos[2], "Wo3": Wos[3],
            "bq": bq, "bk": bk, "bv": bv, "bo": bo,
            "embq": embq, "embk": embk, "onesblk": onesblk_np,
        })
    res = run_bass_kernel_spmd(nc, maps, list(range(NCORES)))
    out = np.concatenate([res.results[c]["out"] for c in range(NCORES)], axis=0)
    return out.astype(np.float32)
